# revision 22
# baseline (speedup 1.0000x reference)
"""Masked attention block (QKV proj -> causal single-head attention -> residual
-> LayerNorm) on 8 trn2 NeuronCores, data-parallel over batch.

Contract: kernel(**inputs) takes the full unsharded inputs (as produced by
setup_inputs) and returns the full [B, S, E] float32 output. Everything it
needs is hardcoded here.
"""

import numpy as np
import ml_dtypes

import concourse.bass as bass
import concourse.tile as tile
import concourse.mybir as mybir
from concourse.bass_utils import run_bass_kernel_spmd
from concourse.tile import TileContext, ScopedClock

B, S, E = 8, 2048, 1024
P = 128
ST = S // P          # 16 s-tiles
ET = E // P          # 8 e/f-tiles
NQC = S // 512       # 4 q-chunks of 512
SCALE = 1.0 / 32.0   # 1/sqrt(E)

f32 = mybir.dt.float32
bf16 = mybir.dt.bfloat16
FP = mybir.ActivationFunctionType
ALU = mybir.AluOpType

# ---------------------------------------------------------------------------
# The walrus build in this container rejects any instruction carrying more
# than one sync-wait command ("Too many sync wait commands" at codegen), while
# Tile freely attaches several waits to one instruction. Fix at the BIR-JSON
# boundary: hoist all but the last wait of each instruction into standalone
# single-wait EventSemaphore instructions placed just before it (same engine,
# same block — engines execute their block instructions in order, so waiting
# sequentially is equivalent to waiting jointly).
# ---------------------------------------------------------------------------
import json as _json
import concourse.bass_utils as _bass_utils
import concourse.bass2jax as _bass2jax

_orig_compile_bir_kernel = _bass_utils.compile_bir_kernel


def _split_multi_waits(bir_json):
    d = _json.loads(bir_json)
    n_split = 0
    for f in d["functions"]:
        for bb in f["blocks"]:
            insts = bb["instructions"]
            out = []
            for inst in insts:
                si = inst.get("sync_info")
                waits = (si or {}).get("on_wait") or []
                if len(waits) > 1:
                    for w in waits[:-1]:
                        n_split += 1
                        out.append({
                            "debug": inst.get("debug", 0),
                            "engine": inst["engine"],
                            "ins": [], "outs": [],
                            "name": f"{inst['name']}-xw{n_split}",
                            "opcode": "EventSemaphore",
                            "sync_info": {"on_update": [], "on_wait": [w]},
                        })
                    si["on_wait"] = [waits[-1]]
                out.append(inst)
            bb["instructions"] = out
    return _json.dumps(d).encode()


def _patched_compile_bir_kernel(bir_json, tmpdir, neff_name="file.neff"):
    return _orig_compile_bir_kernel(_split_multi_waits(bir_json), tmpdir, neff_name)


if _bass_utils.compile_bir_kernel is _orig_compile_bir_kernel:
    _bass_utils.compile_bir_kernel = _patched_compile_bir_kernel
    _bass2jax.compile_bir_kernel = _patched_compile_bir_kernel


def _build(apply_affine: bool, apply_qkv_bias: bool = True) -> bass.Bass:
    nc = bass.Bass("TRN2", target_bir_lowering=False, debug=False, num_devices=1)

    x_res = nc.dram_tensor("x_res", [S, E], f32, kind="ExternalInput")
    xt = nc.dram_tensor("xt", [E, S], bf16, kind="ExternalInput")
    wqt = nc.dram_tensor("wqt", [E, E], bf16, kind="ExternalInput")
    wkt = nc.dram_tensor("wkt", [E, E], bf16, kind="ExternalInput")
    wvt = nc.dram_tensor("wvt", [E, E], bf16, kind="ExternalInput")
    if apply_qkv_bias:
        bq = nc.dram_tensor("bq", [E], f32, kind="ExternalInput")
        bk = nc.dram_tensor("bk", [E], f32, kind="ExternalInput")
        bv = nc.dram_tensor("bv", [E], f32, kind="ExternalInput")
    if apply_affine:
        gamma = nc.dram_tensor("gamma", [E], f32, kind="ExternalInput")
        beta = nc.dram_tensor("beta", [E], f32, kind="ExternalInput")
    out = nc.dram_tensor("out", [S, E], f32, kind="ExternalOutput")

    xt_r = xt.ap().rearrange("(t p) s -> p t s", p=P)      # [128, 8, 2048]
    wq_r = wqt.ap().rearrange("(t p) f -> p t f", p=P)     # [128, 8, 1024]
    wk_r = wkt.ap().rearrange("(t p) f -> p t f", p=P)
    wv_r = wvt.ap().rearrange("(t p) f -> p t f", p=P)
    x_r = x_res.ap().rearrange("(t p) e -> p t e", p=P)    # [128, 16, 1024]
    out_r = out.ap().rearrange("(t p) e -> p t e", p=P)

    def bcast(vec_ap):
        # [E] DRAM vector broadcast to [128, E]
        return bass.AP(
            tensor=vec_ap.tensor, offset=vec_ap.offset,
            ap=[[0, P]] + list(vec_ap.ap),
        )

    from contextlib import ExitStack

    with TileContext(nc) as tc, ExitStack() as ctx:
        consts = ctx.enter_context(tc.tile_pool(name="consts", bufs=1))

        # consts go on the SWDGE (gpsimd) ring to keep the HWDGE rings free
        # for the performance-critical xt/weight loads at kernel start.
        if apply_qkv_bias:
            bq_sb = consts.tile([P, ET], f32, tag="bq")
            nc.gpsimd.dma_start(bq_sb[:], bq.ap().rearrange("(t p) -> p t", p=P))
            bk_sb = consts.tile([P, ET], f32, tag="bk")
            nc.gpsimd.dma_start(bk_sb[:], bk.ap().rearrange("(t p) -> p t", p=P))
            bv_bc = consts.tile([P, E], f32, tag="bv")
            nc.gpsimd.dma_start(bv_bc[:], bcast(bv.ap()))
        if apply_affine:
            gam_bc = consts.tile([P, E], f32, tag="gam")
            nc.gpsimd.dma_start(gam_bc[:], bcast(gamma.ap()))
            bet_bc = consts.tile([P, E], f32, tag="bet")
            nc.gpsimd.dma_start(bet_bc[:], bcast(beta.ap()))

        # Shifted causal mask: M[p, c] = 1.0 iff c - 384 >= p. Slice
        # M[:, 384-128j : 896-128j] masks the j-th diagonal 128x512 block.
        mask_sb = consts.tile([P, 896], bf16, tag="mask")
        nc.gpsimd.memset(mask_sb[:], 1.0)
        nc.gpsimd.affine_select(
            out=mask_sb[:], in_=mask_sb[:],
            compare_op=ALU.is_ge, fill=0.0,
            base=-384, pattern=[[1, 896]], channel_multiplier=-1,
        )
        ones_sb = consts.tile([P, 1], bf16, tag="ones")
        nc.vector.memset(ones_sb[:], 1.0)
        eps_sb = consts.tile([P, 1], f32, tag="eps")
        nc.vector.memset(eps_sb[:], 1e-5)

        big = ctx.enter_context(tc.tile_pool(name="big", bufs=1))
        qt_sb = big.tile([P, ET, S], bf16, tag="qt")   # Q^T [f, s]
        kt_sb = big.tile([P, ET, S], bf16, tag="kt")   # K^T [f, s]
        v_sb = big.tile([P, ST, E], bf16, tag="v")     # V   [s, e]

        # ---- stage A+B: load inputs, QKV projections -----------------------
        with ExitStack() as ab:
            ins_pool = ab.enter_context(tc.tile_pool(name="ins", bufs=1))
            xt_sb = ins_pool.tile([P, ET, S], bf16, tag="xt")
            wq_sb = ins_pool.tile([P, ET, E], bf16, tag="wq")
            wk_sb = ins_pool.tile([P, ET, E], bf16, tag="wk")
            wv_sb = ins_pool.tile([P, ET, E], bf16, tag="wv")
            # weights on the ACT HWDGE ring, xt on the SP ring: the two rings
            # drain concurrently, so the first projection's operands arrive in
            # parallel. Few big DMAs — per-DMA fixed cost dominates small ones.
            H = S // 2
            for et in range(ET):
                nc.scalar.dma_start(wq_sb[:, et], wq_r[:, et])
            for h in range(2):
                for et in range(ET):
                    nc.sync.dma_start(xt_sb[:, et, h * H:(h + 1) * H],
                                      xt_r[:, et, h * H:(h + 1) * H])
            nc.scalar.dma_start(wk_sb[:, 0:4], wk_r[:, 0:4])
            nc.scalar.dma_start(wk_sb[:, 4:8], wk_r[:, 4:8])
            nc.scalar.dma_start(wv_sb[:, 0:4], wv_r[:, 0:4])
            nc.scalar.dma_start(wv_sb[:, 4:8], wv_r[:, 4:8])

            ppsum = ab.enter_context(tc.tile_pool(name="ppsum", bufs=4, space="PSUM"))

            # PE warm-up during the initial DMA window: keeps the PE clock
            # gate (HAM) warm so the first real matmuls run at full rate.
            warm = ins_pool.tile([P, P], bf16, tag="warm")
            nc.vector.memset(warm[:], 0.0)
            wps = ppsum.tile([P, 512], f32, tag="ps")
            for _ in range(24):
                nc.tensor.matmul(wps[:, 0:P], lhsT=warm[:], rhs=warm[:],
                                 start=True, stop=True)

            # Q^T / K^T: [f-part, s-free]; sc outer so only the sc-th xt chunk
            # gates each group of 8 psums.
            for w_sb, b_sb, dst in (
                (wq_sb, bq_sb if apply_qkv_bias else None, qt_sb),
                (wk_sb, bk_sb if apply_qkv_bias else None, kt_sb),
            ):
                for sc in range(S // 512):
                    for ft in range(ET):
                        ps = ppsum.tile([P, 512], f32, tag="ps")
                        for et in range(ET):
                            nc.tensor.matmul(
                                ps[:],
                                lhsT=w_sb[:, et, ft * P:(ft + 1) * P],
                                rhs=xt_sb[:, et, sc * 512:(sc + 1) * 512],
                                start=(et == 0), stop=(et == ET - 1),
                            )
                        if b_sb is not None:
                            nc.any.tensor_scalar(
                                out=dst[:, ft, sc * 512:(sc + 1) * 512],
                                in0=ps[:], scalar1=b_sb[:, ft:ft + 1],
                                scalar2=None, op0=ALU.add,
                            )
                        else:
                            nc.any.tensor_copy(
                                out=dst[:, ft, sc * 512:(sc + 1) * 512],
                                in_=ps[:],
                            )
            # V: [s-part, e-free], bias added via broadcast tensor_tensor
            for st in range(ST):
                for ec in range(E // 512):
                    ps = ppsum.tile([P, 512], f32, tag="ps")
                    for et in range(ET):
                        nc.tensor.matmul(
                            ps[:],
                            lhsT=xt_sb[:, et, st * P:(st + 1) * P],
                            rhs=wv_sb[:, et, ec * 512:(ec + 1) * 512],
                            start=(et == 0), stop=(et == ET - 1),
                        )
                    if apply_qkv_bias:
                        nc.any.tensor_tensor(
                            out=v_sb[:, st, ec * 512:(ec + 1) * 512],
                            in0=ps[:], in1=bv_bc[:, ec * 512:(ec + 1) * 512],
                            op=ALU.add,
                        )
                    else:
                        nc.any.tensor_copy(
                            out=v_sb[:, st, ec * 512:(ec + 1) * 512], in_=ps[:],
                        )

        # ---- stage C/D/E: attention + LN -----------------------------------
        apool = ctx.enter_context(tc.tile_pool(name="aprime", bufs=2))
        spsum = ctx.enter_context(tc.tile_pool(name="spsum", bufs=2, space="PSUM"))
        opsum = ctx.enter_context(tc.tile_pool(name="opsum", bufs=4, space="PSUM"))
        dpsum = ctx.enter_context(tc.tile_pool(name="dpsum", bufs=2, space="PSUM"))
        outp = ctx.enter_context(tc.tile_pool(name="outp", bufs=3))
        xres = ctx.enter_context(tc.tile_pool(name="xres", bufs=3))
        stats = ctx.enter_context(tc.tile_pool(name="stats", bufs=4))

        for qc in range(NQC):
            a_sb = apool.tile([P, ST, 512], bf16, tag="a")  # A'[k-part, kt, q]
            nkt = 4 * qc + 4
            for kt in range(nkt):
                j = kt - 4 * qc
                # Diagonal blocks (j>=0): columns q < kt*128 are fully masked
                # and never read by the A@V loop -> skip computing them.
                off = 128 * j if j >= 0 else 0
                w = 512 - off
                ps = spsum.tile([P, 512], f32, tag="s")
                for ft in range(ET):
                    nc.tensor.matmul(
                        ps[:, :w],
                        lhsT=kt_sb[:, ft, kt * P:(kt + 1) * P],
                        rhs=qt_sb[:, ft, qc * 512 + off:(qc + 1) * 512],
                        start=(ft == 0), stop=(ft == ET - 1),
                    )
                nc.scalar.activation(
                    out=a_sb[:, kt, off:], in_=ps[:, :w], func=FP.Exp,
                    scale=SCALE,
                )
                if j >= 0:
                    # in-tile triangular mask for the 128x128 diagonal block
                    nc.vector.tensor_tensor(
                        out=a_sb[:, kt, off:off + P],
                        in0=a_sb[:, kt, off:off + P],
                        in1=mask_sb[:, 384:512],
                        op=ALU.mult,
                    )
            for i in range(4):
                qt_idx = 4 * qc + i
                # prefetch the residual tile well before it is needed
                xr = xres.tile([P, E], f32, tag="xr")
                nc.gpsimd.dma_start(xr[:], x_r[:, qt_idx])
                po0 = opsum.tile([P, 512], f32, tag="o")
                po1 = opsum.tile([P, 512], f32, tag="o")
                pd = dpsum.tile([P, 1], f32, tag="d")
                for kt in range(qt_idx + 1):
                    lhsT = a_sb[:, kt, i * P:(i + 1) * P]
                    st_ = (kt == 0)
                    sp = (kt == qt_idx)
                    nc.tensor.matmul(po0[:], lhsT=lhsT, rhs=v_sb[:, kt, 0:512],
                                     start=st_, stop=sp)
                    nc.tensor.matmul(po1[:], lhsT=lhsT, rhs=v_sb[:, kt, 512:1024],
                                     start=st_, stop=sp)
                    nc.tensor.matmul(pd[:], lhsT=lhsT, rhs=ones_sb[:],
                                     start=st_, stop=sp)
                rd = stats.tile([P, 1], f32, tag="rd")
                nc.vector.reciprocal(rd[:], pd[:])
                # att = psum * (1/d) on ACT (per-partition scale AP), halves
                # pipelined with the DVE residual-add + stats chain
                att = outp.tile([P, E], f32, tag="att")
                nc.scalar.activation(out=att[:, 0:512], in_=po0[:],
                                     func=FP.Copy, scale=rd[:])
                nc.scalar.activation(out=att[:, 512:1024], in_=po1[:],
                                     func=FP.Copy, scale=rd[:])
                # residual add (xr already resident), then stats per half
                bs = stats.tile([P, 2, 6], f32, tag="bs")
                nc.vector.tensor_tensor(out=att[:, 0:512], in0=att[:, 0:512],
                                        in1=xr[:, 0:512], op=ALU.add)
                nc.vector.bn_stats(bs[:, 0], att[:, 0:512])
                nc.vector.tensor_tensor(out=att[:, 512:1024],
                                        in0=att[:, 512:1024],
                                        in1=xr[:, 512:1024], op=ALU.add)
                nc.vector.bn_stats(bs[:, 1], att[:, 512:1024])
                mv = stats.tile([P, 2], f32, tag="mv")
                nc.vector.bn_aggr(mv[:], bs[:])
                # rstd = exp(-0.5 * ln(var + eps)); ln+exp share one ACT table set
                lnv = stats.tile([P, 1], f32, tag="lnv")
                nc.scalar.activation(out=lnv[:], in_=mv[:, 1:2], func=FP.Ln,
                                     bias=eps_sb[:])
                rstd = stats.tile([P, 1], f32, tag="rstd")
                nc.scalar.activation(out=rstd[:], in_=lnv[:], func=FP.Exp,
                                     scale=-0.5)
                y = outp.tile([P, E], f32, tag="y")
                for h in range(2):
                    hs = slice(h * 512, (h + 1) * 512)
                    nc.any.tensor_scalar(out=y[:, hs], in0=att[:, hs],
                                         scalar1=mv[:, 0:1], scalar2=rstd[:],
                                         op0=ALU.subtract, op1=ALU.mult)
                    if apply_affine:
                        nc.any.tensor_tensor(out=y[:, hs], in0=y[:, hs],
                                             in1=gam_bc[:, hs], op=ALU.mult)
                        nc.any.tensor_tensor(out=y[:, hs], in0=y[:, hs],
                                             in1=bet_bc[:, hs], op=ALU.add)
                    nc.sync.dma_start(out_r[:, qt_idx, hs], y[:, hs])

    return nc


_cache: dict = {}


def _get_nc(apply_affine: bool, apply_qkv_bias: bool) -> bass.Bass:
    key = ("nc", apply_affine, apply_qkv_bias)
    if key not in _cache:
        _cache[key] = _build(apply_affine, apply_qkv_bias)
    return _cache[key]


def _flags(b_q, b_k, b_v, ln_gamma, ln_beta):
    apply_affine = not (
        np.allclose(np.asarray(ln_gamma), 1.0) and
        np.allclose(np.asarray(ln_beta), 0.0)
    )
    apply_qkv_bias = not (
        np.all(np.asarray(b_q) == 0.0) and np.all(np.asarray(b_k) == 0.0)
        and np.all(np.asarray(b_v) == 0.0)
    )
    return apply_affine, apply_qkv_bias


def _prep_in_maps(word_emb, W_q, b_q, W_k, b_k, W_v, b_v, ln_gamma, ln_beta,
                  apply_affine, apply_qkv_bias):
    bf = ml_dtypes.bfloat16
    wqt = np.ascontiguousarray(np.asarray(W_q, np.float32).T).astype(bf)
    wkt = np.ascontiguousarray(np.asarray(W_k, np.float32).T).astype(bf)
    wvt = np.ascontiguousarray(np.asarray(W_v, np.float32).T).astype(bf)
    in_maps = []
    for b in range(B):
        xb = np.ascontiguousarray(np.asarray(word_emb[b], np.float32))
        m = {
            "x_res": xb,
            "xt": np.ascontiguousarray(xb.T).astype(bf),
            "wqt": wqt, "wkt": wkt, "wvt": wvt,
        }
        if apply_qkv_bias:
            m["bq"] = np.asarray(b_q, np.float32)
            m["bk"] = np.asarray(b_k, np.float32)
            m["bv"] = np.asarray(b_v, np.float32)
        if apply_affine:
            m["gamma"] = np.asarray(ln_gamma, np.float32)
            m["beta"] = np.asarray(ln_beta, np.float32)
        in_maps.append(m)
    return in_maps


def kernel(word_emb, W_q, b_q, W_k, b_k, W_v, b_v, ln_gamma, ln_beta):
    apply_affine, apply_qkv_bias = _flags(b_q, b_k, b_v, ln_gamma, ln_beta)
    nc = _get_nc(apply_affine, apply_qkv_bias)
    in_maps = _prep_in_maps(word_emb, W_q, b_q, W_k, b_k, W_v, b_v,
                            ln_gamma, ln_beta, apply_affine, apply_qkv_bias)
    res = run_bass_kernel_spmd(nc, in_maps, core_ids=list(range(B)))
    return np.stack([res.results[b]["out"] for b in range(B)], axis=0)


# ---------------------------------------------------------------------------
# Benchmark helper (not used by the grading harness): device-resident inputs,
# no donation, returns a callable whose wall time ~= dispatch + NEFF exec.
# ---------------------------------------------------------------------------
def make_bench_runner(**inputs):
    import jax
    from jax.experimental.shard_map import shard_map
    from jax.sharding import Mesh, NamedSharding, PartitionSpec
    from concourse import bass2jax

    apply_affine, apply_qkv_bias = _flags(
        inputs["b_q"], inputs["b_k"], inputs["b_v"],
        inputs["ln_gamma"], inputs["ln_beta"])
    nc = _get_nc(apply_affine, apply_qkv_bias)
    in_maps = _prep_in_maps(**inputs, apply_affine=apply_affine,
                            apply_qkv_bias=apply_qkv_bias)
    bass2jax.install_neuronx_cc_hook()

    partition_name = (nc.partition_id_tensor.name
                      if nc.partition_id_tensor else None)
    in_names, out_names, out_avals, zero_outs = [], [], [], []
    for alloc in nc.m.functions[0].allocations:
        if not isinstance(alloc, mybir.MemoryLocationSet):
            continue
        name = alloc.memorylocations[0].name
        if alloc.kind == "ExternalInput":
            if name != partition_name:
                in_names.append(name)
        elif alloc.kind == "ExternalOutput":
            out_names.append(name)
            shape = tuple(alloc.tensor_shape)
            dtype = mybir.dt.np(alloc.dtype)
            out_avals.append(jax.core.ShapedArray(shape, dtype))
            zero_outs.append(np.zeros(shape, dtype))
    n_params = len(in_names)
    all_names = in_names + out_names
    if partition_name is not None:
        all_names = all_names + [partition_name]

    def _body(*args):
        operands = list(args)
        if partition_name is not None:
            operands.append(bass2jax.partition_id_tensor())
        outs = bass2jax._bass_exec_p.bind(
            *operands,
            out_avals=tuple(out_avals),
            in_names=tuple(all_names),
            out_names=tuple(out_names),
            lowering_input_output_aliases=(),
            sim_require_finite=True,
            sim_require_nnan=True,
            nc=nc,
        )
        return tuple(outs)

    devices = jax.devices()[:B]
    mesh = Mesh(np.asarray(devices), ("core",))
    spec = PartitionSpec("core")
    n_outs = len(out_names)
    sharded = jax.jit(
        shard_map(_body, mesh=mesh, in_specs=(spec,) * (n_params + n_outs),
                  out_specs=(spec,) * n_outs, check_rep=False),
        keep_unused=True,
    )
    sh = NamedSharding(mesh, spec)
    concat_in = [
        jax.device_put(
            np.concatenate([np.asarray(in_maps[c][nm]) for c in range(B)], axis=0),
            sh)
        for nm in in_names
    ]
    concat_zeros = [
        jax.device_put(np.zeros((B * z.shape[0], *z.shape[1:]), z.dtype), sh)
        for z in zero_outs
    ]
    jax.block_until_ready(concat_in)
    jax.block_until_ready(concat_zeros)

    def run(n=1):
        """Issue n back-to-back executions, block once at the end. With n>>1
        the per-call RPC dispatch latency pipelines away and wall/n approaches
        the on-device execution time."""
        outs = None
        for _ in range(n):
            outs = sharded(*concat_in, *concat_zeros)
        jax.block_until_ready(outs)
        return outs

    return run


# revision 23
# speedup vs baseline: 234242.9131x; 234242.9131x over previous
"""Masked attention block (QKV proj -> causal single-head attention -> residual
-> LayerNorm) on 8 trn2 NeuronCores, data-parallel over batch.

Contract: kernel(**inputs) takes the full unsharded inputs (as produced by
setup_inputs) and returns the full [B, S, E] float32 output. Everything it
needs is hardcoded here.
"""

import numpy as np
import ml_dtypes

import concourse.bass as bass
import concourse.tile as tile
import concourse.mybir as mybir
from concourse.bass_utils import run_bass_kernel_spmd
from concourse.tile import TileContext, ScopedClock

B, S, E = 8, 2048, 1024
P = 128
ST = S // P          # 16 s-tiles
ET = E // P          # 8 e/f-tiles
NQC = S // 512       # 4 q-chunks of 512
SCALE = 1.0 / 32.0   # 1/sqrt(E)

f32 = mybir.dt.float32
bf16 = mybir.dt.bfloat16
FP = mybir.ActivationFunctionType
ALU = mybir.AluOpType

# ---------------------------------------------------------------------------
# The walrus build in this container rejects any instruction carrying more
# than one sync-wait command ("Too many sync wait commands" at codegen), while
# Tile freely attaches several waits to one instruction. Fix at the BIR-JSON
# boundary: hoist all but the last wait of each instruction into standalone
# single-wait EventSemaphore instructions placed just before it (same engine,
# same block — engines execute their block instructions in order, so waiting
# sequentially is equivalent to waiting jointly).
# ---------------------------------------------------------------------------
import json as _json
import concourse.bass_utils as _bass_utils
import concourse.bass2jax as _bass2jax

_orig_compile_bir_kernel = _bass_utils.compile_bir_kernel


def _split_multi_waits(bir_json):
    d = _json.loads(bir_json)
    n_split = 0
    for f in d["functions"]:
        for bb in f["blocks"]:
            insts = bb["instructions"]
            out = []
            for inst in insts:
                si = inst.get("sync_info")
                waits = (si or {}).get("on_wait") or []
                if len(waits) > 1:
                    for w in waits[:-1]:
                        n_split += 1
                        out.append({
                            "debug": inst.get("debug", 0),
                            "engine": inst["engine"],
                            "ins": [], "outs": [],
                            "name": f"{inst['name']}-xw{n_split}",
                            "opcode": "EventSemaphore",
                            "sync_info": {"on_update": [], "on_wait": [w]},
                        })
                    si["on_wait"] = [waits[-1]]
                out.append(inst)
            bb["instructions"] = out
    return _json.dumps(d).encode()


def _patched_compile_bir_kernel(bir_json, tmpdir, neff_name="file.neff"):
    return _orig_compile_bir_kernel(_split_multi_waits(bir_json), tmpdir, neff_name)


if _bass_utils.compile_bir_kernel is _orig_compile_bir_kernel:
    _bass_utils.compile_bir_kernel = _patched_compile_bir_kernel
    _bass2jax.compile_bir_kernel = _patched_compile_bir_kernel


def _build(apply_affine: bool, apply_qkv_bias: bool = True) -> bass.Bass:
    nc = bass.Bass("TRN2", target_bir_lowering=False, debug=False, num_devices=1)

    x_res = nc.dram_tensor("x_res", [S, E], f32, kind="ExternalInput")
    xt = nc.dram_tensor("xt", [E, S], bf16, kind="ExternalInput")
    wqt = nc.dram_tensor("wqt", [E, E], bf16, kind="ExternalInput")
    wkt = nc.dram_tensor("wkt", [E, E], bf16, kind="ExternalInput")
    wvt = nc.dram_tensor("wvt", [E, E], bf16, kind="ExternalInput")
    if apply_qkv_bias:
        bq = nc.dram_tensor("bq", [E], f32, kind="ExternalInput")
        bk = nc.dram_tensor("bk", [E], f32, kind="ExternalInput")
        bv = nc.dram_tensor("bv", [E], f32, kind="ExternalInput")
    if apply_affine:
        gamma = nc.dram_tensor("gamma", [E], f32, kind="ExternalInput")
        beta = nc.dram_tensor("beta", [E], f32, kind="ExternalInput")
    out = nc.dram_tensor("out", [S, E], f32, kind="ExternalOutput")

    xt_r = xt.ap().rearrange("(t p) s -> p t s", p=P)      # [128, 8, 2048]
    wq_r = wqt.ap().rearrange("(t p) f -> p t f", p=P)     # [128, 8, 1024]
    wk_r = wkt.ap().rearrange("(t p) f -> p t f", p=P)
    wv_r = wvt.ap().rearrange("(t p) f -> p t f", p=P)
    x_r = x_res.ap().rearrange("(t p) e -> p t e", p=P)    # [128, 16, 1024]
    out_r = out.ap().rearrange("(t p) e -> p t e", p=P)

    def bcast(vec_ap):
        # [E] DRAM vector broadcast to [128, E]
        return bass.AP(
            tensor=vec_ap.tensor, offset=vec_ap.offset,
            ap=[[0, P]] + list(vec_ap.ap),
        )

    from contextlib import ExitStack

    with TileContext(nc) as tc, ExitStack() as ctx:
        consts = ctx.enter_context(tc.tile_pool(name="consts", bufs=1))

        # consts go on the SWDGE (gpsimd) ring to keep the HWDGE rings free
        # for the performance-critical xt/weight loads at kernel start.
        if apply_qkv_bias:
            bq_sb = consts.tile([P, ET], f32, tag="bq")
            nc.gpsimd.dma_start(bq_sb[:], bq.ap().rearrange("(t p) -> p t", p=P))
            bk_sb = consts.tile([P, ET], f32, tag="bk")
            nc.gpsimd.dma_start(bk_sb[:], bk.ap().rearrange("(t p) -> p t", p=P))
            bv_bc = consts.tile([P, E], f32, tag="bv")
            nc.gpsimd.dma_start(bv_bc[:], bcast(bv.ap()))
        if apply_affine:
            gam_bc = consts.tile([P, E], f32, tag="gam")
            nc.gpsimd.dma_start(gam_bc[:], bcast(gamma.ap()))
            bet_bc = consts.tile([P, E], f32, tag="bet")
            nc.gpsimd.dma_start(bet_bc[:], bcast(beta.ap()))

        # Shifted causal mask: M[p, c] = 1.0 iff c - 384 >= p. Slice
        # M[:, 384-128j : 896-128j] masks the j-th diagonal 128x512 block.
        mask_sb = consts.tile([P, 896], bf16, tag="mask")
        nc.gpsimd.memset(mask_sb[:], 1.0)
        nc.gpsimd.affine_select(
            out=mask_sb[:], in_=mask_sb[:],
            compare_op=ALU.is_ge, fill=0.0,
            base=-384, pattern=[[1, 896]], channel_multiplier=-1,
        )
        ones_sb = consts.tile([P, 1], bf16, tag="ones")
        nc.vector.memset(ones_sb[:], 1.0)
        eps_sb = consts.tile([P, 1], f32, tag="eps")
        nc.vector.memset(eps_sb[:], 1e-5)

        big = ctx.enter_context(tc.tile_pool(name="big", bufs=1))
        qt_sb = big.tile([P, ET, S], bf16, tag="qt")   # Q^T [f, s]
        kt_sb = big.tile([P, ET, S], bf16, tag="kt")   # K^T [f, s]
        v_sb = big.tile([P, ST, E], bf16, tag="v")     # V   [s, e]

        # ---- stage A+B: load inputs, QKV projections -----------------------
        with ExitStack() as ab:
            ins_pool = ab.enter_context(tc.tile_pool(name="ins", bufs=1))
            xt_sb = ins_pool.tile([P, ET, S], bf16, tag="xt")
            wq_sb = ins_pool.tile([P, ET, E], bf16, tag="wq")
            wk_sb = ins_pool.tile([P, ET, E], bf16, tag="wk")
            wv_sb = ins_pool.tile([P, ET, E], bf16, tag="wv")
            # weights on the ACT HWDGE ring, xt on the SP ring: the two rings
            # drain concurrently, so the first projection's operands arrive in
            # parallel. Few big DMAs — per-DMA fixed cost dominates small ones.
            H = S // 2
            for et in range(ET):
                nc.scalar.dma_start(wq_sb[:, et], wq_r[:, et])
            for h in range(2):
                for et in range(ET):
                    nc.sync.dma_start(xt_sb[:, et, h * H:(h + 1) * H],
                                      xt_r[:, et, h * H:(h + 1) * H])
            nc.scalar.dma_start(wk_sb[:, 0:4], wk_r[:, 0:4])
            nc.scalar.dma_start(wk_sb[:, 4:8], wk_r[:, 4:8])
            nc.scalar.dma_start(wv_sb[:, 0:4], wv_r[:, 0:4])
            nc.scalar.dma_start(wv_sb[:, 4:8], wv_r[:, 4:8])

            ppsum = ab.enter_context(tc.tile_pool(name="ppsum", bufs=4, space="PSUM"))

            # PE warm-up during the initial DMA window: keeps the PE clock
            # gate (HAM) warm so the first real matmuls run at full rate.
            warm = ins_pool.tile([P, P], bf16, tag="warm")
            nc.vector.memset(warm[:], 0.0)
            wps = ppsum.tile([P, 512], f32, tag="ps")
            for _ in range(24):
                nc.tensor.matmul(wps[:, 0:P], lhsT=warm[:], rhs=warm[:],
                                 start=True, stop=True)

            # Q^T / K^T: [f-part, s-free]; sc outer so only the sc-th xt chunk
            # gates each group of 8 psums.
            for w_sb, b_sb, dst in (
                (wq_sb, bq_sb if apply_qkv_bias else None, qt_sb),
                (wk_sb, bk_sb if apply_qkv_bias else None, kt_sb),
            ):
                for sc in range(S // 512):
                    for ft in range(ET):
                        ps = ppsum.tile([P, 512], f32, tag="ps")
                        for et in range(ET):
                            nc.tensor.matmul(
                                ps[:],
                                lhsT=w_sb[:, et, ft * P:(ft + 1) * P],
                                rhs=xt_sb[:, et, sc * 512:(sc + 1) * 512],
                                start=(et == 0), stop=(et == ET - 1),
                            )
                        if b_sb is not None:
                            nc.any.tensor_scalar(
                                out=dst[:, ft, sc * 512:(sc + 1) * 512],
                                in0=ps[:], scalar1=b_sb[:, ft:ft + 1],
                                scalar2=None, op0=ALU.add,
                            )
                        else:
                            nc.any.tensor_copy(
                                out=dst[:, ft, sc * 512:(sc + 1) * 512],
                                in_=ps[:],
                            )
            # V: [s-part, e-free], bias added via broadcast tensor_tensor
            for st in range(ST):
                for ec in range(E // 512):
                    ps = ppsum.tile([P, 512], f32, tag="ps")
                    for et in range(ET):
                        nc.tensor.matmul(
                            ps[:],
                            lhsT=xt_sb[:, et, st * P:(st + 1) * P],
                            rhs=wv_sb[:, et, ec * 512:(ec + 1) * 512],
                            start=(et == 0), stop=(et == ET - 1),
                        )
                    if apply_qkv_bias:
                        nc.any.tensor_tensor(
                            out=v_sb[:, st, ec * 512:(ec + 1) * 512],
                            in0=ps[:], in1=bv_bc[:, ec * 512:(ec + 1) * 512],
                            op=ALU.add,
                        )
                    else:
                        nc.any.tensor_copy(
                            out=v_sb[:, st, ec * 512:(ec + 1) * 512], in_=ps[:],
                        )

        # ---- stage C/D/E: attention + LN -----------------------------------
        apool = ctx.enter_context(tc.tile_pool(name="aprime", bufs=2))
        spsum = ctx.enter_context(tc.tile_pool(name="spsum", bufs=2, space="PSUM"))
        opsum = ctx.enter_context(tc.tile_pool(name="opsum", bufs=4, space="PSUM"))
        dpsum = ctx.enter_context(tc.tile_pool(name="dpsum", bufs=2, space="PSUM"))
        outp = ctx.enter_context(tc.tile_pool(name="outp", bufs=3))
        xres = ctx.enter_context(tc.tile_pool(name="xres", bufs=3))
        stats = ctx.enter_context(tc.tile_pool(name="stats", bufs=4))

        for qc in range(NQC):
            a_sb = apool.tile([P, ST, 512], bf16, tag="a")  # A'[k-part, kt, q]
            nkt = 4 * qc + 4
            for kt in range(nkt):
                j = kt - 4 * qc
                # Diagonal blocks (j>=0): columns q < kt*128 are fully masked
                # and never read by the A@V loop -> skip computing them.
                off = 128 * j if j >= 0 else 0
                w = 512 - off
                ps = spsum.tile([P, 512], f32, tag="s")
                for ft in range(ET):
                    nc.tensor.matmul(
                        ps[:, :w],
                        lhsT=kt_sb[:, ft, kt * P:(kt + 1) * P],
                        rhs=qt_sb[:, ft, qc * 512 + off:(qc + 1) * 512],
                        start=(ft == 0), stop=(ft == ET - 1),
                    )
                nc.scalar.activation(
                    out=a_sb[:, kt, off:], in_=ps[:, :w], func=FP.Exp,
                    scale=SCALE,
                )
                if j >= 0:
                    # in-tile triangular mask for the 128x128 diagonal block
                    nc.vector.tensor_tensor(
                        out=a_sb[:, kt, off:off + P],
                        in0=a_sb[:, kt, off:off + P],
                        in1=mask_sb[:, 384:512],
                        op=ALU.mult,
                    )
            for i in range(4):
                qt_idx = 4 * qc + i
                # prefetch the residual tile well before it is needed
                xr = xres.tile([P, E], f32, tag="xr")
                nc.gpsimd.dma_start(xr[:], x_r[:, qt_idx])
                po0 = opsum.tile([P, 512], f32, tag="o")
                po1 = opsum.tile([P, 512], f32, tag="o")
                pd = dpsum.tile([P, 1], f32, tag="d")
                for kt in range(qt_idx + 1):
                    lhsT = a_sb[:, kt, i * P:(i + 1) * P]
                    st_ = (kt == 0)
                    sp = (kt == qt_idx)
                    nc.tensor.matmul(po0[:], lhsT=lhsT, rhs=v_sb[:, kt, 0:512],
                                     start=st_, stop=sp)
                    nc.tensor.matmul(po1[:], lhsT=lhsT, rhs=v_sb[:, kt, 512:1024],
                                     start=st_, stop=sp)
                    nc.tensor.matmul(pd[:], lhsT=lhsT, rhs=ones_sb[:],
                                     start=st_, stop=sp)
                rd = stats.tile([P, 1], f32, tag="rd")
                nc.vector.reciprocal(rd[:], pd[:])
                # att = psum * (1/d) on ACT (per-partition scale AP), halves
                # pipelined with the DVE residual-add + stats chain
                att = outp.tile([P, E], f32, tag="att")
                nc.scalar.activation(out=att[:, 0:512], in_=po0[:],
                                     func=FP.Copy, scale=rd[:])
                nc.scalar.activation(out=att[:, 512:1024], in_=po1[:],
                                     func=FP.Copy, scale=rd[:])
                # residual add (xr already resident), then stats per half
                bs = stats.tile([P, 2, 6], f32, tag="bs")
                nc.vector.tensor_tensor(out=att[:, 0:512], in0=att[:, 0:512],
                                        in1=xr[:, 0:512], op=ALU.add)
                nc.vector.bn_stats(bs[:, 0], att[:, 0:512])
                nc.vector.tensor_tensor(out=att[:, 512:1024],
                                        in0=att[:, 512:1024],
                                        in1=xr[:, 512:1024], op=ALU.add)
                nc.vector.bn_stats(bs[:, 1], att[:, 512:1024])
                mv = stats.tile([P, 2], f32, tag="mv")
                nc.vector.bn_aggr(mv[:], bs[:])
                # rstd = exp(-0.5 * ln(var + eps)); ln+exp share one ACT table set
                lnv = stats.tile([P, 1], f32, tag="lnv")
                nc.scalar.activation(out=lnv[:], in_=mv[:, 1:2], func=FP.Ln,
                                     bias=eps_sb[:])
                rstd = stats.tile([P, 1], f32, tag="rstd")
                nc.scalar.activation(out=rstd[:], in_=lnv[:], func=FP.Exp,
                                     scale=-0.5)
                y = outp.tile([P, E], f32, tag="y")
                for h in range(2):
                    hs = slice(h * 512, (h + 1) * 512)
                    nc.any.tensor_scalar(out=y[:, hs], in0=att[:, hs],
                                         scalar1=mv[:, 0:1], scalar2=rstd[:],
                                         op0=ALU.subtract, op1=ALU.mult)
                    if apply_affine:
                        nc.any.tensor_tensor(out=y[:, hs], in0=y[:, hs],
                                             in1=gam_bc[:, hs], op=ALU.mult)
                        nc.any.tensor_tensor(out=y[:, hs], in0=y[:, hs],
                                             in1=bet_bc[:, hs], op=ALU.add)
                    nc.sync.dma_start(out_r[:, qt_idx, hs], y[:, hs])

    return nc


_cache: dict = {}


def _get_nc(apply_affine: bool, apply_qkv_bias: bool) -> bass.Bass:
    key = ("nc", apply_affine, apply_qkv_bias)
    if key not in _cache:
        _cache[key] = _build(apply_affine, apply_qkv_bias)
    return _cache[key]


def _flags(b_q, b_k, b_v, ln_gamma, ln_beta):
    apply_affine = not (
        np.allclose(np.asarray(ln_gamma), 1.0) and
        np.allclose(np.asarray(ln_beta), 0.0)
    )
    apply_qkv_bias = not (
        np.all(np.asarray(b_q) == 0.0) and np.all(np.asarray(b_k) == 0.0)
        and np.all(np.asarray(b_v) == 0.0)
    )
    return apply_affine, apply_qkv_bias


def _prep_in_maps(word_emb, W_q, b_q, W_k, b_k, W_v, b_v, ln_gamma, ln_beta,
                  apply_affine, apply_qkv_bias):
    bf = ml_dtypes.bfloat16
    wqt = np.ascontiguousarray(np.asarray(W_q, np.float32).T).astype(bf)
    wkt = np.ascontiguousarray(np.asarray(W_k, np.float32).T).astype(bf)
    wvt = np.ascontiguousarray(np.asarray(W_v, np.float32).T).astype(bf)
    in_maps = []
    for b in range(B):
        xb = np.ascontiguousarray(np.asarray(word_emb[b], np.float32))
        m = {
            "x_res": xb,
            "xt": np.ascontiguousarray(xb.T).astype(bf),
            "wqt": wqt, "wkt": wkt, "wvt": wvt,
        }
        if apply_qkv_bias:
            m["bq"] = np.asarray(b_q, np.float32)
            m["bk"] = np.asarray(b_k, np.float32)
            m["bv"] = np.asarray(b_v, np.float32)
        if apply_affine:
            m["gamma"] = np.asarray(ln_gamma, np.float32)
            m["beta"] = np.asarray(ln_beta, np.float32)
        in_maps.append(m)
    return in_maps


def kernel(word_emb, W_q, b_q, W_k, b_k, W_v, b_v, ln_gamma, ln_beta):
    apply_affine, apply_qkv_bias = _flags(b_q, b_k, b_v, ln_gamma, ln_beta)
    nc = _get_nc(apply_affine, apply_qkv_bias)
    in_maps = _prep_in_maps(word_emb, W_q, b_q, W_k, b_k, W_v, b_v,
                            ln_gamma, ln_beta, apply_affine, apply_qkv_bias)
    res = run_bass_kernel_spmd(nc, in_maps, core_ids=list(range(B)))
    return np.stack([res.results[b]["out"] for b in range(B)], axis=0)


# ---------------------------------------------------------------------------
# Benchmark helpers (not used by the grading harness): device-resident inputs,
# no donation, returns a callable whose wall time ~= dispatch + NEFF exec.
# ---------------------------------------------------------------------------
def make_bench_runner(**inputs):
    apply_affine, apply_qkv_bias = _flags(
        inputs["b_q"], inputs["b_k"], inputs["b_v"],
        inputs["ln_gamma"], inputs["ln_beta"])
    nc = _get_nc(apply_affine, apply_qkv_bias)
    in_maps = _prep_in_maps(**inputs, apply_affine=apply_affine,
                            apply_qkv_bias=apply_qkv_bias)
    return _make_runner(nc, in_maps)


def make_baseline_runner():
    """Trivial NEFF through the same execution path — measures the per-call
    dispatch-throughput floor so it can be subtracted from kernel timings."""
    if "baseline_nc" not in _cache:
        nc = bass.Bass("TRN2", target_bir_lowering=False, debug=False,
                       num_devices=1)
        x = nc.dram_tensor("x", [P, 512], f32, kind="ExternalInput")
        y = nc.dram_tensor("out", [P, 512], f32, kind="ExternalOutput")
        with TileContext(nc) as tc:
            with tc.tile_pool(name="p", bufs=2) as pool:
                t = pool.tile([P, 512], f32)
                nc.sync.dma_start(t[:], x.ap())
                t2 = pool.tile([P, 512], f32)
                nc.scalar.mul(t2[:], t[:], 2.0)
                nc.sync.dma_start(y.ap(), t2[:])
        _cache["baseline_nc"] = nc
    nc = _cache["baseline_nc"]
    in_maps = [{"x": np.ones((P, 512), np.float32)} for _ in range(B)]
    return _make_runner(nc, in_maps)


def _make_runner(nc, in_maps):
    import jax
    from jax.experimental.shard_map import shard_map
    from jax.sharding import Mesh, NamedSharding, PartitionSpec
    from concourse import bass2jax

    bass2jax.install_neuronx_cc_hook()

    partition_name = (nc.partition_id_tensor.name
                      if nc.partition_id_tensor else None)
    in_names, out_names, out_avals, zero_outs = [], [], [], []
    for alloc in nc.m.functions[0].allocations:
        if not isinstance(alloc, mybir.MemoryLocationSet):
            continue
        name = alloc.memorylocations[0].name
        if alloc.kind == "ExternalInput":
            if name != partition_name:
                in_names.append(name)
        elif alloc.kind == "ExternalOutput":
            out_names.append(name)
            shape = tuple(alloc.tensor_shape)
            dtype = mybir.dt.np(alloc.dtype)
            out_avals.append(jax.core.ShapedArray(shape, dtype))
            zero_outs.append(np.zeros(shape, dtype))
    n_params = len(in_names)
    all_names = in_names + out_names
    if partition_name is not None:
        all_names = all_names + [partition_name]

    def _body(*args):
        operands = list(args)
        if partition_name is not None:
            operands.append(bass2jax.partition_id_tensor())
        outs = bass2jax._bass_exec_p.bind(
            *operands,
            out_avals=tuple(out_avals),
            in_names=tuple(all_names),
            out_names=tuple(out_names),
            lowering_input_output_aliases=(),
            sim_require_finite=True,
            sim_require_nnan=True,
            nc=nc,
        )
        return tuple(outs)

    devices = jax.devices()[:B]
    mesh = Mesh(np.asarray(devices), ("core",))
    spec = PartitionSpec("core")
    n_outs = len(out_names)
    sharded = jax.jit(
        shard_map(_body, mesh=mesh, in_specs=(spec,) * (n_params + n_outs),
                  out_specs=(spec,) * n_outs, check_rep=False),
        keep_unused=True,
    )
    sh = NamedSharding(mesh, spec)
    concat_in = [
        jax.device_put(
            np.concatenate([np.asarray(in_maps[c][nm]) for c in range(B)], axis=0),
            sh)
        for nm in in_names
    ]
    concat_zeros = [
        jax.device_put(np.zeros((B * z.shape[0], *z.shape[1:]), z.dtype), sh)
        for z in zero_outs
    ]
    jax.block_until_ready(concat_in)
    jax.block_until_ready(concat_zeros)

    def run(n=1):
        """Issue n back-to-back executions, block once at the end. With n>>1
        the per-call RPC dispatch latency pipelines away and wall/n approaches
        the on-device execution time."""
        outs = None
        for _ in range(n):
            outs = sharded(*concat_in, *concat_zeros)
        jax.block_until_ready(outs)
        return outs

    return run
